# revision 14
# baseline (speedup 1.0000x reference)
"""Trainium2 Bass kernel for nn_CGRegressorAdapter (GNN message passing).

Strategy:
  - Data-parallel over B=32 graphs: 8 cores x 4 graphs each. Weights replicated.
  - Per-graph dense adjacency AT[src, dst] (edge-count matrix) built on host
    from edge_index (pure integer layout prep), shipped bf16 (counts are exact).
  - All node states kept transposed [128 feat, 2048 nodes] in f32.
  - GraphConv: m = h @ Wnbr via f32 PE matmuls; m split into bf16 hi+lo;
    agg^T accumulated as (m_hi^T + m_lo^T) @ AT rows streamed 512-wide (bf16 PE),
    plus f32 Wself path, all into the same PSUM; fused bias+ReLU on ACT.
  - Last-node extraction via one-hot column mask + DVE multiply-reduce.
  - Small regression head entirely on-chip in f32.
"""
import numpy as np
import ml_dtypes

import concourse.bass as bass
import concourse.mybir as mybir
from concourse import bacc
from concourse.bass import ts
from concourse.bass_utils import run_bass_kernel_spmd
from concourse.tile import TileContext

BF16 = ml_dtypes.bfloat16
FP8 = ml_dtypes.float8_e4m3
F32 = np.float32

B, N, E, H, L, VOCAB = 32, 2048, 8192, 128, 4, 32
N_CORES = 8
NG = B // N_CORES          # graphs per core
NJ = N // 128              # 16 src chunks
NSPAN = N // 512           # 4 psum spans
dt = mybir.dt
Alu = mybir.AluOpType
Act = mybir.ActivationFunctionType

# bias column indices in the packed bias tile
BCOL_BASE = 0      # 0..3  base_b
BCOL_ADAPT = 4     # 4..7  adapt_b
BCOL_HB1 = 8
BCOL_HMID = 9      # 9..11
BCOL_HB5 = 12
NBCOL = 16


def _build_program(n_graphs=NG, l_base=L, l_adapt=L, do_head=True, n_repeat=1):
    nc = bacc.Bacc("TRN2", target_bir_lowering=False, debug=False,
                   num_devices=N_CORES)
    f32, bf16, fp8 = dt.float32, dt.bfloat16, dt.float8e4

    at_d = nc.declare_dram_parameter("at", [NG * NJ, 128, N], fp8, isOutput=False)
    erhs_d = nc.declare_dram_parameter("embed_rhs", [NG, 128, N], f32, isOutput=False)
    sel_d = nc.declare_dram_parameter("selrep", [NG, 128, N], bf16, isOutput=False)
    embw_d = nc.declare_dram_parameter("embed_w", [128, H], f32, isOutput=False)
    bws_d = nc.declare_dram_parameter("bwself", [L, H, H], f32, isOutput=False)
    bwn_d = nc.declare_dram_parameter("bwnbr", [L, H, H], f32, isOutput=False)
    aws_d = nc.declare_dram_parameter("awself", [L, H, 2, H], f32, isOutput=False)
    awn_d = nc.declare_dram_parameter("awnbr", [L, H, 2, H], f32, isOutput=False)
    hw1_d = nc.declare_dram_parameter("hw1", [H, 2, H], f32, isOutput=False)
    hwm_d = nc.declare_dram_parameter("hwmid", [H, 3, H], f32, isOutput=False)
    hw5_d = nc.declare_dram_parameter("hw5", [H, 1], f32, isOutput=False)
    bias_d = nc.declare_dram_parameter("biases", [H, NBCOL], f32, isOutput=False)
    y_d = nc.declare_dram_parameter("y", [1, NG], f32, isOutput=True)

    with TileContext(nc) as tc:
        with (
            tc.tile_pool(name="const", bufs=1) as const,
            tc.tile_pool(name="atp", bufs=2) as atp,
            tc.tile_pool(name="state", bufs=1) as state,
            tc.tile_pool(name="currp", bufs=2) as currp,
            tc.tile_pool(name="mp", bufs=4) as mp,
            tc.tile_pool(name="work", bufs=2) as work,
            tc.tile_pool(name="psum_agg", bufs=1, space="PSUM") as psum_agg,
            tc.tile_pool(name="psum_m", bufs=4, space="PSUM") as psum_m,
        ):
            # ---- constants ----
            embw = const.tile([128, H], f32)
            nc.sync.dma_start(embw[:], embw_d[:])
            bias_t = const.tile([H, NBCOL], f32)
            nc.sync.dma_start(bias_t[:], bias_d[:])
            bws_t = []
            bwn_t = []
            aws_t = []
            awn_t = []
            for i in range(L):
                w = const.tile([H, H], f32, tag=f"bws{i}")
                nc.sync.dma_start(w[:], bws_d[i])
                bws_t.append(w)
                w = const.tile([H, H], f32, tag=f"bwn{i}")
                nc.sync.dma_start(w[:], bwn_d[i])
                bwn_t.append(w)
                w = const.tile([H, 2, H], f32, tag=f"aws{i}")
                nc.sync.dma_start(w[:], aws_d[i])
                aws_t.append(w)
                w = const.tile([H, 2, H], f32, tag=f"awn{i}")
                nc.sync.dma_start(w[:], awn_d[i])
                awn_t.append(w)
            hw1_t = const.tile([H, 2, H], f32)
            nc.sync.dma_start(hw1_t[:], hw1_d[:])
            hwm_t = const.tile([H, 3, H], f32)
            nc.sync.dma_start(hwm_t[:], hwm_d[:])
            hw5_t = const.tile([H, 1], f32)
            nc.sync.dma_start(hw5_t[:], hw5_d[:])
            dummy = const.tile([128, 1], f32)

            gbT = state.tile([128, NG], f32, tag="gb")
            gaT = state.tile([128, NG], f32, tag="ga")

            def gconv(nbr_srcs, self_srcs, at_tiles, bias_col, out_tile,
                      relu=True):
                """nbr_srcs: list of (stateT_tile, Wnbr_rhs_ap [128,H]).
                self_srcs: list of (stateT_tile, Wself_lhsT_ap [128,H]).
                out_tile: [128, N] f32 output state."""
                aggs = [psum_agg.tile([128, 512], f32, tag=f"agg{s}",
                                      name=f"agg{s}") for s in range(NSPAN)]

                def emit_m(j):
                    pm = psum_m.tile([128, 128], f32, tag="pm")
                    nlast = len(nbr_srcs) - 1
                    for idx, (src, w) in enumerate(nbr_srcs):
                        nc.tensor.matmul(pm[:], src[:, ts(j, 128)], w,
                                         start=(idx == 0), stop=(idx == nlast))
                    mhi = mp.tile([128, 128], bf16, tag="mhi")
                    nc.scalar.copy(mhi[:], pm[:])
                    return mhi

                m_next = emit_m(0)
                for idx, (src, w) in enumerate(self_srcs):
                    for s in range(NSPAN):
                        nc.tensor.matmul(aggs[s][:], w, src[:, ts(s, 512)],
                                         start=(idx == 0), stop=False)
                for j in range(NJ):
                    mhi = m_next
                    if j + 1 < NJ:
                        m_next = emit_m(j + 1)
                    for s in range(NSPAN):
                        nc.tensor.matmul(aggs[s][:], mhi[:],
                                         at_tiles[j][:, ts(s, 512)],
                                         start=False, stop=(j == NJ - 1))
                func = Act.Relu if relu else Act.Identity
                for s in range(NSPAN):
                    nc.scalar.activation(out_tile[:, ts(s, 512)],
                                         aggs[s][:], func,
                                         bias=bias_t[:, bias_col:bias_col + 1])

            for g in list(range(n_graphs)) * n_repeat:
                # ---- embed + mask DMAs first (critical path for layer 0) ----
                erhs = work.tile([128, N], f32, tag="erhs")
                nc.sync.dma_start(erhs[:], erhs_d[g])
                selt = work.tile([128, N], bf16, tag="sel")
                nc.sync.dma_start(selt[:], sel_d[g])
                at_t = []
                for j in range(NJ):
                    t = atp.tile([128, N], fp8, tag=f"at{j}", name=f"at{j}")
                    nc.sync.dma_start(t[:], at_d[g * NJ + j])
                    at_t.append(t)

                lat = [state.tile([128, N], f32, tag=f"lat{k}", name=f"lat{k}")
                       for k in range(L + 1)]
                eaggs = [psum_agg.tile([128, 512], f32, tag=f"agg{s}",
                                       name=f"eagg{s}") for s in range(NSPAN)]
                for s in range(NSPAN):
                    nc.tensor.matmul(eaggs[s][:], embw[:],
                                     erhs[:, ts(s, 512)], start=True, stop=True)
                for s in range(NSPAN):
                    nc.scalar.copy(lat[0][:, ts(s, 512)], eaggs[s][:])

                # ---- base stack ----
                for i in range(l_base):
                    gconv(nbr_srcs=[(lat[i], bwn_t[i][:])],
                          self_srcs=[(lat[i], bws_t[i][:])],
                          at_tiles=at_t, bias_col=BCOL_BASE + i,
                          out_tile=lat[i + 1])

                # ---- adapter stack ----
                curr = lat[0]
                for i in range(l_adapt):
                    ncurr = currp.tile([128, N], f32, tag="curr")
                    gconv(nbr_srcs=[(lat[i + 1], awn_t[i][:, 0, :]),
                                    (curr, awn_t[i][:, 1, :])],
                          self_srcs=[(lat[i + 1], aws_t[i][:, 0, :]),
                                     (curr, aws_t[i][:, 1, :])],
                          at_tiles=at_t, bias_col=BCOL_ADAPT + i,
                          out_tile=ncurr)
                    curr = ncurr

                # ---- last-node extraction (mask-multiply + reduce) ----
                extr = work.tile([128, N], f32, tag="extr")
                nc.vector.tensor_mul(out=extr[:], in0=lat[l_base][:], in1=selt[:])
                nc.vector.tensor_reduce(gbT[:, g:g + 1], extr[:],
                                        mybir.AxisListType.X, Alu.add)
                extr2 = work.tile([128, N], f32, tag="extr")
                nc.vector.tensor_mul(out=extr2[:], in0=curr[:], in1=selt[:])
                nc.vector.tensor_reduce(gaT[:, g:g + 1], extr2[:],
                                        mybir.AxisListType.X, Alu.add)

            if do_head:
                # ---- regression head (all graphs at once) ----
                def head_mm(lhsT, rhs, bias_col, func):
                    pm = psum_m.tile([128, 128], f32, tag="pm")
                    nc.tensor.matmul(pm[:, :NG], lhsT, rhs, start=True, stop=True)
                    out = state.tile([128, NG], f32, tag="hy")
                    nc.scalar.activation(out[:], pm[:, :NG], func,
                                         bias=bias_t[:, bias_col:bias_col + 1])
                    return out

                pm = psum_m.tile([128, 128], f32, tag="pm")
                nc.tensor.matmul(pm[:, :NG], hw1_t[:, 0, :], gbT[:], start=True, stop=False)
                nc.tensor.matmul(pm[:, :NG], hw1_t[:, 1, :], gaT[:], start=False, stop=True)
                y1 = state.tile([128, NG], f32, tag="hy")
                nc.scalar.activation(y1[:], pm[:, :NG], Act.Identity,
                                     bias=bias_t[:, BCOL_HB1:BCOL_HB1 + 1])
                y2 = head_mm(hwm_t[:, 0, :], y1[:], BCOL_HMID + 0, Act.Relu)
                y3 = head_mm(hwm_t[:, 1, :], y2[:], BCOL_HMID + 1, Act.Identity)
                y4 = head_mm(hwm_t[:, 2, :], y3[:], BCOL_HMID + 2, Act.Relu)
                pm5 = psum_m.tile([128, 128], f32, tag="pm")
                nc.tensor.matmul(pm5[:1, :NG], hw5_t[:], y4[:], start=True, stop=True)
                yout = state.tile([1, NG], f32, tag="yout")
                nc.scalar.activation(yout[:], pm5[:1, :NG], Act.Identity,
                                     bias=bias_t[:1, BCOL_HB5:BCOL_HB5 + 1])
                nc.sync.dma_start(y_d[:], yout[:])
            else:
                yout = state.tile([1, NG], f32, tag="yout")
                nc.vector.tensor_copy(out=yout[:], in_=gbT[:1, :])
                nc.sync.dma_start(y_d[:], yout[:])

    nc.compile()
    return nc


_NC_CACHE = {}


def _get_program():
    if "nc" not in _NC_CACHE:
        _NC_CACHE["nc"] = _build_program()
    return _NC_CACHE["nc"]


def _prep_inputs(inputs):
    """Host-side sharding + layout prep. Returns list of per-core in_maps."""
    inds = np.asarray(inputs["regular_node_inds"]).astype(np.int64)
    shapes = np.asarray(inputs["regular_node_shapes"], dtype=F32)
    edge = np.asarray(inputs["edge_index"]).astype(np.int64)
    last_idx = np.asarray(inputs["last_idx"]).astype(np.int64)

    # adjacency AT[src, dst] counts per graph, fp8 e4m3 (exact small ints)
    at_all = np.zeros((B, N, N), dtype=F32)
    for g in range(B):
        np.add.at(at_all[g], (edge[g, 0], edge[g, 1]), 1.0)
    at_all = at_all.astype(FP8)

    # embed rhs: rows 0..31 one-hot(inds)^T, rows 32..35 shapes^T, rest 0
    erhs_all = np.zeros((B, 128, N), dtype=F32)
    ar = np.arange(N)
    for g in range(B):
        erhs_all[g, inds[g], ar] = 1.0
        erhs_all[g, VOCAB:VOCAB + 4, :] = shapes[g].T
    # last-node selection mask replicated over partitions
    sel_all = np.zeros((B, 128, N), dtype=BF16)
    for g in range(B):
        sel_all[g, :, last_idx[g]] = 1.0

    embed_w = np.zeros((128, H), dtype=F32)
    embed_w[:VOCAB] = np.asarray(inputs["embed_table"], dtype=F32)
    embed_w[VOCAB:VOCAB + 4] = np.asarray(inputs["shape_w"], dtype=F32)

    aws = np.asarray(inputs["adapt_Wself"], dtype=F32).reshape(L, 2, H, H)
    awn = np.asarray(inputs["adapt_Wnbr"], dtype=F32).reshape(L, 2, H, H)
    aws = np.ascontiguousarray(aws.transpose(0, 2, 1, 3))  # [L, H, 2, H]
    awn = np.ascontiguousarray(awn.transpose(0, 2, 1, 3))
    hw1 = np.ascontiguousarray(
        np.asarray(inputs["hW1"], dtype=F32).reshape(2, H, H).transpose(1, 0, 2))

    biases = np.zeros((H, NBCOL), dtype=F32)
    biases[:, BCOL_BASE:BCOL_BASE + L] = np.asarray(inputs["base_b"], dtype=F32).T
    biases[:, BCOL_ADAPT:BCOL_ADAPT + L] = np.asarray(inputs["adapt_b"], dtype=F32).T
    biases[:, BCOL_HB1] = np.asarray(inputs["hb1"], dtype=F32)
    biases[:, BCOL_HMID:BCOL_HMID + 3] = np.asarray(inputs["hbmid"], dtype=F32).T
    biases[0, BCOL_HB5] = np.asarray(inputs["hb5"], dtype=F32)[0]

    shared = {
        "embed_w": embed_w,
        "bwself": np.asarray(inputs["base_Wself"], dtype=F32),
        "bwnbr": np.asarray(inputs["base_Wnbr"], dtype=F32),
        "awself": aws,
        "awnbr": awn,
        "hw1": hw1,
        "hwmid": np.ascontiguousarray(
            np.asarray(inputs["hWmid"], dtype=F32).transpose(1, 0, 2)),
        "hw5": np.asarray(inputs["hW5"], dtype=F32),
        "biases": biases,
    }
    in_maps = []
    for c in range(N_CORES):
        g0 = c * NG
        in_maps.append({
            "at": np.ascontiguousarray(
                at_all[g0:g0 + NG].reshape(NG * NJ, 128, N)),
            "embed_rhs": erhs_all[g0:g0 + NG],
            "selrep": sel_all[g0:g0 + NG],
            **shared,
        })
    return in_maps


def kernel(**inputs) -> np.ndarray:
    nc = _get_program()
    in_maps = _prep_inputs(inputs)
    res = run_bass_kernel_spmd(nc, in_maps, core_ids=list(range(N_CORES)))
    out = np.concatenate([res.results[c]["y"].reshape(NG) for c in range(N_CORES)])
    return out.reshape(B, 1).astype(F32)



# revision 21
# speedup vs baseline: 2.4808x; 2.4808x over previous
"""Trainium2 Bass kernel for nn_CGRegressorAdapter (GNN message passing).

Strategy:
  - Data-parallel over B=32 graphs: 8 cores x 4 graphs each. Weights replicated.
  - Per-graph dense adjacency AT[src, dst] (edge-count matrix) built on host
    from edge_index (pure integer layout prep), shipped bf16 (counts are exact).
  - All node states kept transposed [128 feat, 2048 nodes] in f32.
  - GraphConv: m = h @ Wnbr via f32 PE matmuls; m split into bf16 hi+lo;
    agg^T accumulated as (m_hi^T + m_lo^T) @ AT rows streamed 512-wide (bf16 PE),
    plus f32 Wself path, all into the same PSUM; fused bias+ReLU on ACT.
  - Last-node extraction via one-hot column mask + DVE multiply-reduce.
  - Small regression head entirely on-chip in f32.
"""
import numpy as np
import ml_dtypes

import concourse.bass as bass
import concourse.mybir as mybir
from concourse import bacc
from concourse.bass import ts
from concourse.bass_utils import run_bass_kernel_spmd
from concourse.tile import TileContext

BF16 = ml_dtypes.bfloat16
FP8 = ml_dtypes.float8_e4m3
F32 = np.float32

B, N, E, H, L, VOCAB = 32, 2048, 8192, 128, 4, 32
N_CORES = 8
NG = B // N_CORES          # graphs per core
NJ = N // 128              # 16 src chunks
NSPAN = N // 512           # 4 psum spans
dt = mybir.dt
Alu = mybir.AluOpType
Act = mybir.ActivationFunctionType

# bias column indices in the packed bias tile
BCOL_BASE = 0      # 0..3  base_b
BCOL_ADAPT = 4     # 4..7  adapt_b
BCOL_HB1 = 8
BCOL_HMID = 9      # 9..11
BCOL_HB5 = 12
NBCOL = 16


def _build_program(n_graphs=NG, l_base=L, l_adapt=L, do_head=True, n_repeat=1):
    nc = bacc.Bacc("TRN2", target_bir_lowering=False, debug=False,
                   num_devices=N_CORES)
    f32, bf16, fp8 = dt.float32, dt.bfloat16, dt.float8e4

    at_d = nc.declare_dram_parameter("at", [NG * NJ, 128, N], fp8, isOutput=False)
    erhs_d = nc.declare_dram_parameter("embed_rhs", [NG, 128, N], bf16, isOutput=False)
    sel_d = nc.declare_dram_parameter("selrep", [NG, 128, N], bf16, isOutput=False)
    embw_d = nc.declare_dram_parameter("embed_w", [128, H], bf16, isOutput=False)
    bws_d = nc.declare_dram_parameter("bwself", [L, H, H], bf16, isOutput=False)
    bwn_d = nc.declare_dram_parameter("bwnbr", [L, H, H], bf16, isOutput=False)
    aws_d = nc.declare_dram_parameter("awself", [L, H, 2, H], bf16, isOutput=False)
    awn_d = nc.declare_dram_parameter("awnbr", [L, H, 2, H], bf16, isOutput=False)
    hw1_d = nc.declare_dram_parameter("hw1", [H, 2, H], f32, isOutput=False)
    hwm_d = nc.declare_dram_parameter("hwmid", [H, 3, H], f32, isOutput=False)
    hw5_d = nc.declare_dram_parameter("hw5", [H, 1], f32, isOutput=False)
    bias_d = nc.declare_dram_parameter("biases", [H, NBCOL], f32, isOutput=False)
    y_d = nc.declare_dram_parameter("y", [1, NG], f32, isOutput=True)

    with TileContext(nc) as tc:
        with (
            tc.tile_pool(name="const", bufs=1) as const,
            tc.tile_pool(name="atp", bufs=2) as atp,
            tc.tile_pool(name="state", bufs=1) as state,
            tc.tile_pool(name="currp", bufs=2) as currp,
            tc.tile_pool(name="mp", bufs=4) as mp,
            tc.tile_pool(name="work", bufs=2) as work,
            tc.tile_pool(name="psum_agg", bufs=1, space="PSUM") as psum_agg,
            tc.tile_pool(name="psum_m", bufs=4, space="PSUM") as psum_m,
        ):
            # ---- constants ----
            embw = const.tile([128, H], bf16)
            nc.sync.dma_start(embw[:], embw_d[:])
            bias_t = const.tile([H, NBCOL], f32)
            nc.sync.dma_start(bias_t[:], bias_d[:])
            bws_t = []
            bwn_t = []
            aws_t = []
            awn_t = []
            for i in range(L):
                w = const.tile([H, H], bf16, tag=f"bws{i}")
                nc.sync.dma_start(w[:], bws_d[i])
                bws_t.append(w)
                w = const.tile([H, H], bf16, tag=f"bwn{i}")
                nc.sync.dma_start(w[:], bwn_d[i])
                bwn_t.append(w)
                w = const.tile([H, 2, H], bf16, tag=f"aws{i}")
                nc.sync.dma_start(w[:], aws_d[i])
                aws_t.append(w)
                w = const.tile([H, 2, H], bf16, tag=f"awn{i}")
                nc.sync.dma_start(w[:], awn_d[i])
                awn_t.append(w)
            hw1_t = const.tile([H, 2, H], f32)
            nc.sync.dma_start(hw1_t[:], hw1_d[:])
            hwm_t = const.tile([H, 3, H], f32)
            nc.sync.dma_start(hwm_t[:], hwm_d[:])
            hw5_t = const.tile([H, 1], f32)
            nc.sync.dma_start(hw5_t[:], hw5_d[:])
            dummy = const.tile([128, 1], f32)

            gbT = state.tile([128, NG], f32, tag="gb")
            gaT = state.tile([128, NG], f32, tag="ga")

            def gconv(nbr_srcs, self_srcs, at_tiles, bias_col, out_tile,
                      relu=True):
                """nbr_srcs: list of (bf16 stateT_tile, bf16 Wnbr_rhs [128,H]).
                self_srcs: list of (bf16 stateT_tile, bf16 Wself_lhsT [128,H]).
                out_tile: [128, N] bf16 output state."""
                aggs = [psum_agg.tile([128, 512], f32, tag=f"agg{s}",
                                      name=f"agg{s}") for s in range(NSPAN)]

                def emit_m(j):
                    pm = psum_m.tile([128, 128], f32, tag="pm")
                    nlast = len(nbr_srcs) - 1
                    for idx, (src, w) in enumerate(nbr_srcs):
                        nc.tensor.matmul(pm[:], src[:, ts(j, 128)], w,
                                         start=(idx == 0), stop=(idx == nlast))
                    mhi = mp.tile([128, 128], bf16, tag="mhi")
                    nc.scalar.copy(mhi[:], pm[:])
                    return mhi

                m_next = emit_m(0)
                for idx, (src, w) in enumerate(self_srcs):
                    for s in range(NSPAN):
                        nc.tensor.matmul(aggs[s][:], w, src[:, ts(s, 512)],
                                         start=(idx == 0), stop=False)
                for j in range(NJ):
                    mhi = m_next
                    if j + 1 < NJ:
                        m_next = emit_m(j + 1)
                    for s in range(NSPAN):
                        nc.tensor.matmul(aggs[s][:], mhi[:],
                                         at_tiles[j][:, ts(s, 512)],
                                         start=False, stop=(j == NJ - 1))
                func = Act.Relu if relu else Act.Identity
                for s in range(NSPAN):
                    nc.scalar.activation(out_tile[:, ts(s, 512)],
                                         aggs[s][:], func,
                                         bias=bias_t[:, bias_col:bias_col + 1])

            for g in list(range(n_graphs)) * n_repeat:
                # ---- embed + mask DMAs first (critical path for layer 0) ----
                erhs = work.tile([128, N], bf16, tag="erhs")
                nc.sync.dma_start(erhs[:], erhs_d[g])
                selt = work.tile([128, N], bf16, tag="sel")
                nc.sync.dma_start(selt[:], sel_d[g])
                at_t = []
                for j in range(NJ):
                    t = atp.tile([128, N], fp8, tag=f"at{j}", name=f"at{j}")
                    nc.sync.dma_start(t[:], at_d[g * NJ + j])
                    at_t.append(t)

                lat = [state.tile([128, N], bf16, tag=f"lat{k}", name=f"lat{k}")
                       for k in range(L + 1)]
                eaggs = [psum_agg.tile([128, 512], f32, tag=f"agg{s}",
                                       name=f"eagg{s}") for s in range(NSPAN)]
                for s in range(NSPAN):
                    nc.tensor.matmul(eaggs[s][:], embw[:],
                                     erhs[:, ts(s, 512)], start=True, stop=True)
                for s in range(NSPAN):
                    nc.scalar.copy(lat[0][:, ts(s, 512)], eaggs[s][:])

                # ---- base stack ----
                for i in range(l_base):
                    gconv(nbr_srcs=[(lat[i], bwn_t[i][:])],
                          self_srcs=[(lat[i], bws_t[i][:])],
                          at_tiles=at_t, bias_col=BCOL_BASE + i,
                          out_tile=lat[i + 1])

                # ---- adapter stack ----
                curr = lat[0]
                for i in range(l_adapt):
                    ncurr = currp.tile([128, N], bf16, tag="curr")
                    gconv(nbr_srcs=[(lat[i + 1], awn_t[i][:, 0, :]),
                                    (curr, awn_t[i][:, 1, :])],
                          self_srcs=[(lat[i + 1], aws_t[i][:, 0, :]),
                                     (curr, aws_t[i][:, 1, :])],
                          at_tiles=at_t, bias_col=BCOL_ADAPT + i,
                          out_tile=ncurr)
                    curr = ncurr

                # ---- last-node extraction (mask-multiply + reduce) ----
                extr = work.tile([128, N], f32, tag="extr")
                nc.vector.tensor_mul(out=extr[:], in0=lat[l_base][:], in1=selt[:])
                nc.vector.tensor_reduce(gbT[:, g:g + 1], extr[:],
                                        mybir.AxisListType.X, Alu.add)
                extr2 = work.tile([128, N], f32, tag="extr")
                nc.vector.tensor_mul(out=extr2[:], in0=curr[:], in1=selt[:])
                nc.vector.tensor_reduce(gaT[:, g:g + 1], extr2[:],
                                        mybir.AxisListType.X, Alu.add)

            if do_head:
                # ---- regression head (all graphs at once) ----
                def head_mm(lhsT, rhs, bias_col, func):
                    pm = psum_m.tile([128, 128], f32, tag="pm")
                    nc.tensor.matmul(pm[:, :NG], lhsT, rhs, start=True, stop=True)
                    out = state.tile([128, NG], f32, tag="hy")
                    nc.scalar.activation(out[:], pm[:, :NG], func,
                                         bias=bias_t[:, bias_col:bias_col + 1])
                    return out

                pm = psum_m.tile([128, 128], f32, tag="pm")
                nc.tensor.matmul(pm[:, :NG], hw1_t[:, 0, :], gbT[:], start=True, stop=False)
                nc.tensor.matmul(pm[:, :NG], hw1_t[:, 1, :], gaT[:], start=False, stop=True)
                y1 = state.tile([128, NG], f32, tag="hy")
                nc.scalar.activation(y1[:], pm[:, :NG], Act.Identity,
                                     bias=bias_t[:, BCOL_HB1:BCOL_HB1 + 1])
                y2 = head_mm(hwm_t[:, 0, :], y1[:], BCOL_HMID + 0, Act.Relu)
                y3 = head_mm(hwm_t[:, 1, :], y2[:], BCOL_HMID + 1, Act.Identity)
                y4 = head_mm(hwm_t[:, 2, :], y3[:], BCOL_HMID + 2, Act.Relu)
                pm5 = psum_m.tile([128, 128], f32, tag="pm")
                nc.tensor.matmul(pm5[:1, :NG], hw5_t[:], y4[:], start=True, stop=True)
                yout = state.tile([1, NG], f32, tag="yout")
                nc.scalar.activation(yout[:], pm5[:1, :NG], Act.Identity,
                                     bias=bias_t[:1, BCOL_HB5:BCOL_HB5 + 1])
                nc.sync.dma_start(y_d[:], yout[:])
            else:
                yout = state.tile([1, NG], f32, tag="yout")
                nc.vector.tensor_copy(out=yout[:], in_=gbT[:1, :])
                nc.sync.dma_start(y_d[:], yout[:])

    nc.compile()
    return nc


_NC_CACHE = {}


def _get_program():
    if "nc" not in _NC_CACHE:
        _NC_CACHE["nc"] = _build_program()
    return _NC_CACHE["nc"]


def _prep_inputs(inputs):
    """Host-side sharding + layout prep. Returns list of per-core in_maps."""
    inds = np.asarray(inputs["regular_node_inds"]).astype(np.int64)
    shapes = np.asarray(inputs["regular_node_shapes"], dtype=F32)
    edge = np.asarray(inputs["edge_index"]).astype(np.int64)
    last_idx = np.asarray(inputs["last_idx"]).astype(np.int64)

    # adjacency AT[src, dst] counts per graph, fp8 e4m3 (exact small ints)
    at_all = np.zeros((B, N, N), dtype=F32)
    for g in range(B):
        np.add.at(at_all[g], (edge[g, 0], edge[g, 1]), 1.0)
    at_all = at_all.astype(FP8)

    # embed rhs: rows 0..31 one-hot(inds)^T, rows 32..35 shapes^T, rest 0
    erhs_all = np.zeros((B, 128, N), dtype=F32)
    ar = np.arange(N)
    for g in range(B):
        erhs_all[g, inds[g], ar] = 1.0
        erhs_all[g, VOCAB:VOCAB + 4, :] = shapes[g].T
    # last-node selection mask replicated over partitions
    sel_all = np.zeros((B, 128, N), dtype=BF16)
    for g in range(B):
        sel_all[g, :, last_idx[g]] = 1.0

    embed_w = np.zeros((128, H), dtype=F32)
    embed_w[:VOCAB] = np.asarray(inputs["embed_table"], dtype=F32)
    embed_w[VOCAB:VOCAB + 4] = np.asarray(inputs["shape_w"], dtype=F32)

    aws = np.asarray(inputs["adapt_Wself"], dtype=F32).reshape(L, 2, H, H)
    awn = np.asarray(inputs["adapt_Wnbr"], dtype=F32).reshape(L, 2, H, H)
    aws = np.ascontiguousarray(aws.transpose(0, 2, 1, 3))  # [L, H, 2, H]
    awn = np.ascontiguousarray(awn.transpose(0, 2, 1, 3))
    hw1 = np.ascontiguousarray(
        np.asarray(inputs["hW1"], dtype=F32).reshape(2, H, H).transpose(1, 0, 2))

    biases = np.zeros((H, NBCOL), dtype=F32)
    biases[:, BCOL_BASE:BCOL_BASE + L] = np.asarray(inputs["base_b"], dtype=F32).T
    biases[:, BCOL_ADAPT:BCOL_ADAPT + L] = np.asarray(inputs["adapt_b"], dtype=F32).T
    biases[:, BCOL_HB1] = np.asarray(inputs["hb1"], dtype=F32)
    biases[:, BCOL_HMID:BCOL_HMID + 3] = np.asarray(inputs["hbmid"], dtype=F32).T
    biases[0, BCOL_HB5] = np.asarray(inputs["hb5"], dtype=F32)[0]

    shared = {
        "embed_w": embed_w.astype(BF16),
        "bwself": np.asarray(inputs["base_Wself"], dtype=F32).astype(BF16),
        "bwnbr": np.asarray(inputs["base_Wnbr"], dtype=F32).astype(BF16),
        "awself": aws.astype(BF16),
        "awnbr": awn.astype(BF16),
        "hw1": hw1,
        "hwmid": np.ascontiguousarray(
            np.asarray(inputs["hWmid"], dtype=F32).transpose(1, 0, 2)),
        "hw5": np.asarray(inputs["hW5"], dtype=F32),
        "biases": biases,
    }
    in_maps = []
    for c in range(N_CORES):
        g0 = c * NG
        in_maps.append({
            "at": np.ascontiguousarray(
                at_all[g0:g0 + NG].reshape(NG * NJ, 128, N)),
            "embed_rhs": erhs_all[g0:g0 + NG].astype(BF16),
            "selrep": sel_all[g0:g0 + NG],
            **shared,
        })
    return in_maps


def kernel(**inputs) -> np.ndarray:
    nc = _get_program()
    in_maps = _prep_inputs(inputs)
    res = run_bass_kernel_spmd(nc, in_maps, core_ids=list(range(N_CORES)))
    out = np.concatenate([res.results[c]["y"].reshape(NG) for c in range(N_CORES)])
    return out.reshape(B, 1).astype(F32)



# revision 29
# speedup vs baseline: 2.7962x; 1.1271x over previous
"""Trainium2 Bass kernel for nn_CGRegressorAdapter (GNN message passing).

Strategy:
  - Data-parallel over B=32 graphs: 8 cores x 4 graphs each. Weights replicated.
  - Per-graph dense adjacency AT[src, dst] (edge-count matrix) built on host
    from edge_index, shipped fp8 e4m3 (counts <= 2, exact in fp8); all AT
    tiles double-buffered in SBUF so the next graph's DMA hides under compute.
  - All states/weights bf16 (single-pass, no hi/lo split; PSUM accumulates in
    f32; rel err ~7e-3 vs the 2e-2 gate). Moving operands never f32, which
    keeps every matmul at the PE's 1 column/cycle stream rate.
  - GraphConv: m = h @ Wnbr into PSUM, DVE-cast to bf16 (keeps ACT free);
    agg^T accumulated as m_chunk @ AT rows streamed 512-wide into 4 PSUM
    spans, plus the Wself path, all in the same accumulation group; fused
    bias+ReLU on ACT writes the next bf16 state directly.
  - Startup: only embed weights + layer-0 weights DMA'd before graph 0's
    data; remaining constants load behind it.
  - Last-node extraction via one-hot column mask + DVE multiply-reduce.
  - Small regression head entirely on-chip in f32.
  - A DMA-gather + banded-matmul aggregation path (n_gather>0) exists and
    validates in CoreSim, but the dma_gather ucode returns wrong data on
    this runtime, so it is disabled (N_GATHER=0).
"""
import numpy as np
import ml_dtypes

import concourse.bass as bass
import concourse.mybir as mybir
from concourse import bacc
from concourse.bass import ts
from concourse.bass_utils import run_bass_kernel_spmd
from concourse.tile import TileContext

BF16 = ml_dtypes.bfloat16
FP8 = ml_dtypes.float8_e4m3
F32 = np.float32

B, N, E, H, L, VOCAB = 32, 2048, 8192, 128, 4, 32
N_CORES = 8
NG = B // N_CORES          # graphs per core
NJ = N // 128              # 16 src chunks
NSPAN = N // 512           # 4 psum spans
NEC = E // 128             # 64 edge chunks (gather path)
GW = 64                    # fixed banded window width
N_GATHER = 0               # graphs per core on the DMA-gather path

# fixed banded segment schedule: per span, (chunk, r_lo, r_hi, out_off)
SEGW = []
for _s in range(NSPAN):
    _segs = []
    for _c in range(NEC):
        _wlo = 32 * _c - 16
        _lo = max(_wlo, 512 * _s, 0)
        _hi = min(_wlo + GW, 512 * _s + 512, N)
        if _lo < _hi:
            _segs.append((_c, _lo - _wlo, _hi - _wlo, _lo - 512 * _s))
    SEGW.append(_segs)
dt = mybir.dt
Alu = mybir.AluOpType
Act = mybir.ActivationFunctionType

# bias column indices in the packed bias tile
BCOL_BASE = 0      # 0..3  base_b
BCOL_ADAPT = 4     # 4..7  adapt_b
BCOL_HB1 = 8
BCOL_HMID = 9      # 9..11
BCOL_HB5 = 12
NBCOL = 16


def _build_program(n_graphs=NG, l_base=L, l_adapt=L, do_head=True, n_repeat=1,
                   n_gather=N_GATHER):
    nc = bacc.Bacc("TRN2", target_bir_lowering=False, debug=False,
                   num_devices=N_CORES)
    f32, bf16, fp8 = dt.float32, dt.bfloat16, dt.float8e4

    at_d = nc.declare_dram_parameter("at", [NG * NJ, 128, N], fp8, isOutput=False)
    erhs_d = nc.declare_dram_parameter("embed_rhs", [NG, 128, N], bf16, isOutput=False)
    sel_d = nc.declare_dram_parameter("selrep", [NG, 128, N], bf16, isOutput=False)
    embw_d = nc.declare_dram_parameter("embed_w", [128, H], bf16, isOutput=False)
    bws_d = nc.declare_dram_parameter("bwself", [L, H, H], bf16, isOutput=False)
    bwn_d = nc.declare_dram_parameter("bwnbr", [L, H, H], bf16, isOutput=False)
    aws_d = nc.declare_dram_parameter("awself", [L, H, 2, H], bf16, isOutput=False)
    awn_d = nc.declare_dram_parameter("awnbr", [L, H, 2, H], bf16, isOutput=False)
    hw1_d = nc.declare_dram_parameter("hw1", [H, 2, H], f32, isOutput=False)
    hwm_d = nc.declare_dram_parameter("hwmid", [H, 3, H], f32, isOutput=False)
    hw5_d = nc.declare_dram_parameter("hw5", [H, 1], f32, isOutput=False)
    bias_d = nc.declare_dram_parameter("biases", [H, NBCOL], f32, isOutput=False)
    if n_gather > 0:
        gidx_d = nc.declare_dram_parameter("gidx", [NG, 128, E // 16], dt.int16,
                                           isOutput=False)
        rb_d = nc.declare_dram_parameter("rband", [NG, 128, NEC, GW],
                                         dt.float8e4, isOutput=False)
        md_scr = [nc.dram_tensor(f"mscr{i}", [N, H], dt.bfloat16)
                  for i in range(2)]
    y_d = nc.declare_dram_parameter("y", [1, NG], f32, isOutput=True)

    with TileContext(nc) as tc:
        with (
            tc.tile_pool(name="const", bufs=1) as const,
            tc.tile_pool(name="atp", bufs=2) as atp,
            tc.tile_pool(name="state", bufs=1) as state,
            tc.tile_pool(name="currp", bufs=2) as currp,
            tc.tile_pool(name="mp", bufs=4) as mp,
            tc.tile_pool(name="mgp", bufs=2) as mgp,
            tc.tile_pool(name="work", bufs=2) as work,
            tc.tile_pool(name="psum_agg", bufs=1, space="PSUM") as psum_agg,
            tc.tile_pool(name="psum_m", bufs=4, space="PSUM") as psum_m,
        ):
            if n_gather > 0:
                from concourse import library_config
                nc.gpsimd.load_library(library_config.mlp)
            # ---- constants (critical-path first: embed + layer-0 weights) ----
            embw = const.tile([128, H], bf16)
            nc.sync.dma_start(embw[:], embw_d[:])
            bias_t = const.tile([H, NBCOL], f32)
            nc.sync.dma_start(bias_t[:], bias_d[:])
            bws_t = [const.tile([H, H], bf16, tag=f"bws{i}", name=f"bws{i}")
                     for i in range(L)]
            bwn_t = [const.tile([H, H], bf16, tag=f"bwn{i}", name=f"bwn{i}")
                     for i in range(L)]
            aws_t = [const.tile([H, 2, H], bf16, tag=f"aws{i}", name=f"aws{i}")
                     for i in range(L)]
            awn_t = [const.tile([H, 2, H], bf16, tag=f"awn{i}", name=f"awn{i}")
                     for i in range(L)]
            nc.sync.dma_start(bwn_t[0][:], bwn_d[0])
            nc.sync.dma_start(bws_t[0][:], bws_d[0])
            hw1_t = const.tile([H, 2, H], f32)
            hwm_t = const.tile([H, 3, H], f32)
            hw5_t = const.tile([H, 1], f32)

            def load_late_consts():
                for i in range(L):
                    if i > 0:
                        nc.sync.dma_start(bws_t[i][:], bws_d[i])
                        nc.sync.dma_start(bwn_t[i][:], bwn_d[i])
                    nc.sync.dma_start(aws_t[i][:], aws_d[i])
                    nc.sync.dma_start(awn_t[i][:], awn_d[i])
                nc.sync.dma_start(hw1_t[:], hw1_d[:])
                nc.sync.dma_start(hwm_t[:], hwm_d[:])
                nc.sync.dma_start(hw5_t[:], hw5_d[:])

            gbT = state.tile([128, NG], f32, tag="gb")
            gaT = state.tile([128, NG], f32, tag="ga")

            def gconv(nbr_srcs, self_srcs, at_tiles, bias_col, out_tile,
                      relu=True):
                """nbr_srcs: list of (bf16 stateT_tile, bf16 Wnbr_rhs [128,H]).
                self_srcs: list of (bf16 stateT_tile, bf16 Wself_lhsT [128,H]).
                out_tile: [128, N] bf16 output state."""
                aggs = [psum_agg.tile([128, 512], f32, tag=f"agg{s}",
                                      name=f"agg{s}") for s in range(NSPAN)]

                def emit_m(j):
                    pm = psum_m.tile([128, 128], f32, tag="pm")
                    nlast = len(nbr_srcs) - 1
                    for idx, (src, w) in enumerate(nbr_srcs):
                        nc.tensor.matmul(pm[:], src[:, ts(j, 128)], w,
                                         start=(idx == 0), stop=(idx == nlast))
                    mhi = mp.tile([128, 128], bf16, tag="mhi")
                    nc.vector.tensor_copy(out=mhi[:], in_=pm[:])
                    return mhi

                m_next = emit_m(0)
                for idx, (src, w) in enumerate(self_srcs):
                    for s in range(NSPAN):
                        nc.tensor.matmul(aggs[s][:], w, src[:, ts(s, 512)],
                                         start=(idx == 0), stop=False)
                for j in range(NJ):
                    mhi = m_next
                    if j + 1 < NJ:
                        m_next = emit_m(j + 1)
                    for s in range(NSPAN):
                        nc.tensor.matmul(aggs[s][:], mhi[:],
                                         at_tiles[j][:, ts(s, 512)],
                                         start=False, stop=(j == NJ - 1))
                func = Act.Relu if relu else Act.Identity
                for s in range(NSPAN):
                    nc.scalar.activation(out_tile[:, ts(s, 512)],
                                         aggs[s][:], func,
                                         bias=bias_t[:, bias_col:bias_col + 1])

            def gconv_gather(nbr_srcs, self_srcs, idx_t, rb_t, md, bias_col,
                             out_tile, relu=True):
                """Aggregation via DRAM round-trip gather + banded matmuls.
                md: DRAM scratch [N, H] bf16 for this step's messages."""
                # m-phase: all chunks into one contiguous SBUF tile
                mbig = mp.tile([128, NJ, 128], bf16, tag="mbig")
                nlast = len(nbr_srcs) - 1
                for j in range(NJ):
                    pm = psum_m.tile([128, 128], f32, tag="pm")
                    for idx, (src, w) in enumerate(nbr_srcs):
                        nc.tensor.matmul(pm[:], src[:, ts(j, 128)], w,
                                         start=(idx == 0), stop=(idx == nlast))
                    nc.vector.tensor_copy(out=mbig[:, j, :], in_=pm[:])
                # one DMA: SBUF [p, c, f] -> DRAM rows (c*128+p)
                nc.sync.dma_start(
                    md[:].rearrange("(c p) f -> p c f", c=NJ), mbig[:])
                # gather into edge order (sorted by dst)
                mg = mgp.tile([128, NEC, H], bf16, tag="mg")
                nc.gpsimd.dma_gather(mg[:], md[:], idx_t[:], num_idxs=E,
                                     num_idxs_reg=E, elem_size=H)
                func = Act.Relu if relu else Act.Identity
                for s in range(NSPAN):
                    agg = psum_agg.tile([128, 512], f32, tag=f"agg{s}",
                                        name=f"gagg{s}")
                    for idx, (src, w) in enumerate(self_srcs):
                        nc.tensor.matmul(agg[:], w, src[:, ts(s, 512)],
                                         start=(idx == 0), stop=False)
                    nseg = len(SEGW[s])
                    for k, (c, rlo, rhi, olo) in enumerate(SEGW[s]):
                        nc.tensor.matmul(agg[:, olo:olo + (rhi - rlo)],
                                         mg[:, c, :], rb_t[:, c, rlo:rhi],
                                         start=False, stop=(k == nseg - 1))
                    nc.scalar.activation(out_tile[:, ts(s, 512)],
                                         agg[:], func,
                                         bias=bias_t[:, bias_col:bias_col + 1])

            first = True
            for g in list(range(n_graphs)) * n_repeat:
                use_gather = g < n_gather
                # ---- embed + mask DMAs first (critical path for layer 0) ----
                erhs = work.tile([128, N], bf16, tag="erhs")
                nc.sync.dma_start(erhs[:], erhs_d[g])
                selt = work.tile([128, N], bf16, tag="sel")
                nc.sync.dma_start(selt[:], sel_d[g])
                at_t = []
                idx_t = rb_t = None
                if use_gather:
                    idx_t = work.tile([128, E // 16], dt.int16, tag="gidx")
                    nc.sync.dma_start(idx_t[:], gidx_d[g])
                    rb_t = work.tile([128, NEC, GW], fp8, tag="rband")
                    nc.sync.dma_start(rb_t[:], rb_d[g])
                else:
                    for j in range(NJ):
                        t = atp.tile([128, N], fp8, tag=f"at{j}", name=f"at{j}")
                        nc.sync.dma_start(t[:], at_d[g * NJ + j])
                        at_t.append(t)
                if first:
                    load_late_consts()
                    first = False

                lat = [state.tile([128, N], bf16, tag=f"lat{k}", name=f"lat{k}")
                       for k in range(L + 1)]
                eaggs = [psum_agg.tile([128, 512], f32, tag=f"agg{s}",
                                       name=f"eagg{s}") for s in range(NSPAN)]
                for s in range(NSPAN):
                    nc.tensor.matmul(eaggs[s][:], embw[:],
                                     erhs[:, ts(s, 512)], start=True, stop=True)
                for s in range(NSPAN):
                    nc.scalar.copy(lat[0][:, ts(s, 512)], eaggs[s][:])

                def do_gconv(step, nbr_srcs, self_srcs, bias_col, out_tile):
                    if use_gather:
                        gconv_gather(nbr_srcs, self_srcs, idx_t, rb_t,
                                     md_scr[step % 2], bias_col, out_tile)
                    else:
                        gconv(nbr_srcs, self_srcs, at_t, bias_col, out_tile)

                # ---- base stack ----
                for i in range(l_base):
                    do_gconv(i, [(lat[i], bwn_t[i][:])],
                             [(lat[i], bws_t[i][:])],
                             BCOL_BASE + i, lat[i + 1])

                # ---- adapter stack ----
                curr = lat[0]
                for i in range(l_adapt):
                    ncurr = currp.tile([128, N], bf16, tag="curr")
                    do_gconv(l_base + i,
                             [(lat[i + 1], awn_t[i][:, 0, :]),
                              (curr, awn_t[i][:, 1, :])],
                             [(lat[i + 1], aws_t[i][:, 0, :]),
                              (curr, aws_t[i][:, 1, :])],
                             BCOL_ADAPT + i, ncurr)
                    curr = ncurr

                # ---- last-node extraction (mask-multiply + reduce) ----
                extr = work.tile([128, N], f32, tag="extr")
                nc.vector.tensor_mul(out=extr[:], in0=lat[l_base][:], in1=selt[:])
                nc.vector.tensor_reduce(gbT[:, g:g + 1], extr[:],
                                        mybir.AxisListType.X, Alu.add)
                extr2 = work.tile([128, N], f32, tag="extr")
                nc.vector.tensor_mul(out=extr2[:], in0=curr[:], in1=selt[:])
                nc.vector.tensor_reduce(gaT[:, g:g + 1], extr2[:],
                                        mybir.AxisListType.X, Alu.add)

            if do_head:
                # ---- regression head (all graphs at once) ----
                def head_mm(lhsT, rhs, bias_col, func):
                    pm = psum_m.tile([128, 128], f32, tag="pm")
                    nc.tensor.matmul(pm[:, :NG], lhsT, rhs, start=True, stop=True)
                    out = state.tile([128, NG], f32, tag="hy")
                    nc.scalar.activation(out[:], pm[:, :NG], func,
                                         bias=bias_t[:, bias_col:bias_col + 1])
                    return out

                pm = psum_m.tile([128, 128], f32, tag="pm")
                nc.tensor.matmul(pm[:, :NG], hw1_t[:, 0, :], gbT[:], start=True, stop=False)
                nc.tensor.matmul(pm[:, :NG], hw1_t[:, 1, :], gaT[:], start=False, stop=True)
                y1 = state.tile([128, NG], f32, tag="hy")
                nc.scalar.activation(y1[:], pm[:, :NG], Act.Identity,
                                     bias=bias_t[:, BCOL_HB1:BCOL_HB1 + 1])
                y2 = head_mm(hwm_t[:, 0, :], y1[:], BCOL_HMID + 0, Act.Relu)
                y3 = head_mm(hwm_t[:, 1, :], y2[:], BCOL_HMID + 1, Act.Identity)
                y4 = head_mm(hwm_t[:, 2, :], y3[:], BCOL_HMID + 2, Act.Relu)
                pm5 = psum_m.tile([128, 128], f32, tag="pm")
                nc.tensor.matmul(pm5[:1, :NG], hw5_t[:], y4[:], start=True, stop=True)
                yout = state.tile([1, NG], f32, tag="yout")
                nc.scalar.activation(yout[:], pm5[:1, :NG], Act.Identity,
                                     bias=bias_t[:1, BCOL_HB5:BCOL_HB5 + 1])
                nc.sync.dma_start(y_d[:], yout[:])
            else:
                yout = state.tile([1, NG], f32, tag="yout")
                nc.vector.tensor_copy(out=yout[:], in_=gbT[:1, :])
                nc.sync.dma_start(y_d[:], yout[:])

    nc.compile()
    return nc


_NC_CACHE = {}


def _get_program():
    if "nc" not in _NC_CACHE:
        _NC_CACHE["nc"] = _build_program()
    return _NC_CACHE["nc"]


def _prep_inputs(inputs):
    """Host-side sharding + layout prep. Returns list of per-core in_maps."""
    inds = np.asarray(inputs["regular_node_inds"]).astype(np.int64)
    shapes = np.asarray(inputs["regular_node_shapes"], dtype=F32)
    edge = np.asarray(inputs["edge_index"]).astype(np.int64)
    last_idx = np.asarray(inputs["last_idx"]).astype(np.int64)

    # adjacency AT[src, dst] counts per graph, fp8 e4m3 (exact small ints)
    at_all = np.zeros((B, N, N), dtype=F32)
    for g in range(B):
        np.add.at(at_all[g], (edge[g, 0], edge[g, 1]), 1.0)
    at_all = at_all.astype(FP8)

    # embed rhs: rows 0..31 one-hot(inds)^T, rows 32..35 shapes^T, rest 0
    erhs_all = np.zeros((B, 128, N), dtype=F32)
    ar = np.arange(N)
    for g in range(B):
        erhs_all[g, inds[g], ar] = 1.0
        erhs_all[g, VOCAB:VOCAB + 4, :] = shapes[g].T
    # last-node selection mask replicated over partitions
    sel_all = np.zeros((B, 128, N), dtype=BF16)
    for g in range(B):
        sel_all[g, :, last_idx[g]] = 1.0

    embed_w = np.zeros((128, H), dtype=F32)
    embed_w[:VOCAB] = np.asarray(inputs["embed_table"], dtype=F32)
    embed_w[VOCAB:VOCAB + 4] = np.asarray(inputs["shape_w"], dtype=F32)

    aws = np.asarray(inputs["adapt_Wself"], dtype=F32).reshape(L, 2, H, H)
    awn = np.asarray(inputs["adapt_Wnbr"], dtype=F32).reshape(L, 2, H, H)
    aws = np.ascontiguousarray(aws.transpose(0, 2, 1, 3))  # [L, H, 2, H]
    awn = np.ascontiguousarray(awn.transpose(0, 2, 1, 3))
    hw1 = np.ascontiguousarray(
        np.asarray(inputs["hW1"], dtype=F32).reshape(2, H, H).transpose(1, 0, 2))

    biases = np.zeros((H, NBCOL), dtype=F32)
    biases[:, BCOL_BASE:BCOL_BASE + L] = np.asarray(inputs["base_b"], dtype=F32).T
    biases[:, BCOL_ADAPT:BCOL_ADAPT + L] = np.asarray(inputs["adapt_b"], dtype=F32).T
    biases[:, BCOL_HB1] = np.asarray(inputs["hb1"], dtype=F32)
    biases[:, BCOL_HMID:BCOL_HMID + 3] = np.asarray(inputs["hbmid"], dtype=F32).T
    biases[0, BCOL_HB5] = np.asarray(inputs["hb5"], dtype=F32)[0]

    shared = {
        "embed_w": embed_w.astype(BF16),
        "bwself": np.asarray(inputs["base_Wself"], dtype=F32).astype(BF16),
        "bwnbr": np.asarray(inputs["base_Wnbr"], dtype=F32).astype(BF16),
        "awself": aws.astype(BF16),
        "awnbr": awn.astype(BF16),
        "hw1": hw1,
        "hwmid": np.ascontiguousarray(
            np.asarray(inputs["hWmid"], dtype=F32).transpose(1, 0, 2)),
        "hw5": np.asarray(inputs["hW5"], dtype=F32),
        "biases": biases,
    }
    in_maps = []
    for c in range(N_CORES):
        g0 = c * NG
        in_maps.append({
            "at": np.ascontiguousarray(
                at_all[g0:g0 + NG].reshape(NG * NJ, 128, N)),
            "embed_rhs": erhs_all[g0:g0 + NG].astype(BF16),
            "selrep": sel_all[g0:g0 + NG],
            **shared,
        })
    return in_maps


def kernel(**inputs) -> np.ndarray:
    nc = _get_program()
    in_maps = _prep_inputs(inputs)
    res = run_bass_kernel_spmd(nc, in_maps, core_ids=list(range(N_CORES)))
    out = np.concatenate([res.results[c]["y"].reshape(NG) for c in range(N_CORES)])
    return out.reshape(B, 1).astype(F32)



# revision 31
# speedup vs baseline: 11.5870x; 4.1438x over previous
"""Trainium2 Bass kernel for nn_CGRegressorAdapter (GNN message passing).

Strategy:
  - Data-parallel over B=32 graphs: 8 cores x 4 graphs each. Weights replicated.
  - **Backward reachability slicing**: the readout uses ONE node per graph
    (last_idx), so layer k from the end only needs the k-hop in-neighborhood
    S_k of that node (|S_k| ~ 1, 9, 39, 152, 562, 1390 here). Nodes are
    relabeled per graph on the host so each S_k is a PREFIX; every matmul
    then runs on prefix widths only (exact computation, ~13x fewer
    aggregation columns than the full dense form). Prefix widths are padded
    to shared maxima so all 8 cores run one SPMD program; values computed in
    the padding region are garbage-but-unread by construction.
  - Per-graph dense adjacency AT[src, dst] (edge-count matrix) built on host
    in relabeled order, shipped fp8 e4m3 (counts <= 2, exact), only the
    [S5-rows x S4-cols] block that aggregation can touch.
  - All states/weights bf16 (PSUM accumulates f32; rel err ~7e-3 vs the 2e-2
    gate). Moving operands never f32 (f32 moving streams at 1/4 rate).
  - GraphConv: m = h @ Wnbr into PSUM, DVE-cast to bf16; agg^T accumulated
    as m_chunk @ AT rows over the dst prefix in <=512-col PSUM spans, plus
    the Wself path in the same accumulation group; fused bias+ReLU on ACT
    writes the next bf16 state.
  - Readout: relabeled last node is column 0 - extraction is a [128,1] copy.
  - Small regression head entirely on-chip in f32.
"""
import numpy as np
import ml_dtypes

import concourse.bass as bass
import concourse.mybir as mybir
from concourse import bacc
from concourse.bass import ts
from concourse.bass_utils import run_bass_kernel_spmd
from concourse.tile import TileContext

BF16 = ml_dtypes.bfloat16
FP8 = ml_dtypes.float8_e4m3
F32 = np.float32

B, N, E, H, L, VOCAB = 32, 2048, 8192, 128, 4, 32
N_CORES = 8
NG = B // N_CORES          # graphs per core
dt = mybir.dt
Alu = mybir.AluOpType
Act = mybir.ActivationFunctionType

# bias column indices in the packed bias tile
BCOL_BASE = 0      # 0..3  base_b
BCOL_ADAPT = 4     # 4..7  adapt_b
BCOL_HB1 = 8
BCOL_HMID = 9      # 9..11
BCOL_HB5 = 12
NBCOL = 16


def _spans(w):
    """Split width w into <=512-wide PSUM span pieces: [(span, off, width)]."""
    out = []
    off = 0
    while off < w:
        out.append((off // 512, off, min(512, w - off)))
        off += 512
    return out


def _build_program(ns, n_graphs=NG, n_repeat=1):
    """ns = (N0..N5): padded prefix sizes, multiples of 128, N5 = embed width."""
    N0, N1, N2, N3, N4, N5 = ns
    cdiv = lambda x: (x + 127) // 128
    atch = cdiv(N5)          # adjacency src chunks shipped
    atw = N4                 # adjacency dst width shipped
    # (dst_width, src_chunks) per gconv
    base_cfg = [(N4, cdiv(N5)), (N3, cdiv(N4)), (N2, cdiv(N3)), (N1, cdiv(N2))]
    adapt_cfg = [(N3, cdiv(N4)), (N2, cdiv(N3)), (N1, cdiv(N2)), (N0, cdiv(N1))]

    nc = bacc.Bacc("TRN2", target_bir_lowering=False, debug=False,
                   num_devices=N_CORES)
    f32, bf16, fp8 = dt.float32, dt.bfloat16, dt.float8e4

    at_d = nc.declare_dram_parameter("at", [NG, atch, 128, atw], fp8,
                                     isOutput=False)
    erhs_d = nc.declare_dram_parameter("embed_rhs", [NG, 128, N5], bf16,
                                       isOutput=False)
    embw_d = nc.declare_dram_parameter("embed_w", [128, H], bf16, isOutput=False)
    bws_d = nc.declare_dram_parameter("bwself", [L, H, H], bf16, isOutput=False)
    bwn_d = nc.declare_dram_parameter("bwnbr", [L, H, H], bf16, isOutput=False)
    aws_d = nc.declare_dram_parameter("awself", [L, H, 2, H], bf16, isOutput=False)
    awn_d = nc.declare_dram_parameter("awnbr", [L, H, 2, H], bf16, isOutput=False)
    hw1_d = nc.declare_dram_parameter("hw1", [H, 2, H], f32, isOutput=False)
    hwm_d = nc.declare_dram_parameter("hwmid", [H, 3, H], f32, isOutput=False)
    hw5_d = nc.declare_dram_parameter("hw5", [H, 1], f32, isOutput=False)
    bias_d = nc.declare_dram_parameter("biases", [H, NBCOL], f32, isOutput=False)
    y_d = nc.declare_dram_parameter("y", [1, NG], f32, isOutput=True)

    with TileContext(nc) as tc:
        with (
            tc.tile_pool(name="const", bufs=1) as const,
            tc.tile_pool(name="atp", bufs=2) as atp,
            tc.tile_pool(name="state", bufs=1) as state,
            tc.tile_pool(name="currp", bufs=2) as currp,
            tc.tile_pool(name="mp", bufs=4) as mp,
            tc.tile_pool(name="work", bufs=2) as work,
            tc.tile_pool(name="psum_agg", bufs=1, space="PSUM") as psum_agg,
            tc.tile_pool(name="psum_m", bufs=4, space="PSUM") as psum_m,
        ):
            # ---- constants (critical-path first: embed + layer-0 weights) ----
            embw = const.tile([128, H], bf16)
            nc.sync.dma_start(embw[:], embw_d[:])
            bias_t = const.tile([H, NBCOL], f32)
            nc.sync.dma_start(bias_t[:], bias_d[:])
            bws_t = [const.tile([H, H], bf16, tag=f"bws{i}", name=f"bws{i}")
                     for i in range(L)]
            bwn_t = [const.tile([H, H], bf16, tag=f"bwn{i}", name=f"bwn{i}")
                     for i in range(L)]
            aws_t = [const.tile([H, 2, H], bf16, tag=f"aws{i}", name=f"aws{i}")
                     for i in range(L)]
            awn_t = [const.tile([H, 2, H], bf16, tag=f"awn{i}", name=f"awn{i}")
                     for i in range(L)]
            nc.sync.dma_start(bwn_t[0][:], bwn_d[0])
            nc.sync.dma_start(bws_t[0][:], bws_d[0])
            hw1_t = const.tile([H, 2, H], f32)
            hwm_t = const.tile([H, 3, H], f32)
            hw5_t = const.tile([H, 1], f32)

            def load_late_consts():
                for i in range(L):
                    if i > 0:
                        nc.sync.dma_start(bws_t[i][:], bws_d[i])
                        nc.sync.dma_start(bwn_t[i][:], bwn_d[i])
                    nc.sync.dma_start(aws_t[i][:], aws_d[i])
                    nc.sync.dma_start(awn_t[i][:], awn_d[i])
                nc.sync.dma_start(hw1_t[:], hw1_d[:])
                nc.sync.dma_start(hwm_t[:], hwm_d[:])
                nc.sync.dma_start(hw5_t[:], hw5_d[:])

            gbT = state.tile([128, NG], f32, tag="gb")
            gaT = state.tile([128, NG], f32, tag="ga")

            def gconv(nbr_srcs, self_srcs, at_tiles, bias_col, out_tile,
                      dst_w, src_chunks):
                """All operands bf16. Aggregates over dst prefix [0, dst_w)
                from src chunks [0, src_chunks)."""
                pieces = _spans(dst_w)
                aggs = {s: psum_agg.tile([128, 512], f32, tag=f"agg{s}",
                                         name=f"agg{s}")
                        for s, _, _ in pieces}

                def emit_m(j):
                    pm = psum_m.tile([128, 128], f32, tag="pm")
                    nlast = len(nbr_srcs) - 1
                    for idx, (src, w) in enumerate(nbr_srcs):
                        nc.tensor.matmul(pm[:], src[:, ts(j, 128)], w,
                                         start=(idx == 0), stop=(idx == nlast))
                    mhi = mp.tile([128, 128], bf16, tag="mhi")
                    nc.vector.tensor_copy(out=mhi[:], in_=pm[:])
                    return mhi

                m_next = emit_m(0)
                for idx, (src, w) in enumerate(self_srcs):
                    for s, off, wd in pieces:
                        nc.tensor.matmul(aggs[s][:, :wd], w,
                                         src[:, off:off + wd],
                                         start=(idx == 0), stop=False)
                for j in range(src_chunks):
                    mhi = m_next
                    if j + 1 < src_chunks:
                        m_next = emit_m(j + 1)
                    for s, off, wd in pieces:
                        nc.tensor.matmul(aggs[s][:, :wd], mhi[:],
                                         at_tiles[j][:, off:off + wd],
                                         start=False,
                                         stop=(j == src_chunks - 1))
                for s, off, wd in pieces:
                    nc.scalar.activation(out_tile[:, off:off + wd],
                                         aggs[s][:, :wd], Act.Relu,
                                         bias=bias_t[:, bias_col:bias_col + 1])

            first = True
            for g in list(range(n_graphs)) * n_repeat:
                # ---- per-graph data DMAs ----
                erhs = work.tile([128, N5], bf16, tag="erhs")
                nc.sync.dma_start(erhs[:], erhs_d[g])
                at_t = []
                for j in range(atch):
                    t = atp.tile([128, atw], fp8, tag=f"at{j}", name=f"at{j}")
                    nc.sync.dma_start(t[:], at_d[g, j])
                    at_t.append(t)
                if first:
                    load_late_consts()
                    first = False

                lat = [state.tile([128, N5], bf16, tag=f"lat{k}",
                                  name=f"lat{k}") for k in range(L + 1)]
                for s, off, wd in _spans(N5):
                    eagg = psum_agg.tile([128, 512], f32, tag=f"agg{s}",
                                         name=f"eagg{s}")
                    nc.tensor.matmul(eagg[:, :wd], embw[:],
                                     erhs[:, off:off + wd],
                                     start=True, stop=True)
                    nc.scalar.copy(lat[0][:, off:off + wd], eagg[:, :wd])

                # ---- base stack ----
                for i in range(L):
                    dw, sc = base_cfg[i]
                    gconv([(lat[i], bwn_t[i][:])], [(lat[i], bws_t[i][:])],
                          at_t, BCOL_BASE + i, lat[i + 1], dw, sc)

                # ---- adapter stack ----
                curr = lat[0]
                for i in range(L):
                    dw, sc = adapt_cfg[i]
                    ncurr = currp.tile([128, N3], bf16, tag="curr")
                    gconv([(lat[i + 1], awn_t[i][:, 0, :]),
                           (curr, awn_t[i][:, 1, :])],
                          [(lat[i + 1], aws_t[i][:, 0, :]),
                           (curr, aws_t[i][:, 1, :])],
                          at_t, BCOL_ADAPT + i, ncurr, dw, sc)
                    curr = ncurr

                # ---- readout: relabeled last node is column 0 ----
                nc.vector.tensor_copy(out=gbT[:, g:g + 1], in_=lat[L][:, 0:1])
                nc.vector.tensor_copy(out=gaT[:, g:g + 1], in_=curr[:, 0:1])

            # ---- regression head (all graphs at once) ----
            def head_mm(lhsT, rhs, bias_col, func):
                pm = psum_m.tile([128, 128], f32, tag="pm")
                nc.tensor.matmul(pm[:, :NG], lhsT, rhs, start=True, stop=True)
                out = state.tile([128, NG], f32, tag="hy")
                nc.scalar.activation(out[:], pm[:, :NG], func,
                                     bias=bias_t[:, bias_col:bias_col + 1])
                return out

            pm = psum_m.tile([128, 128], f32, tag="pm")
            nc.tensor.matmul(pm[:, :NG], hw1_t[:, 0, :], gbT[:], start=True,
                             stop=False)
            nc.tensor.matmul(pm[:, :NG], hw1_t[:, 1, :], gaT[:], start=False,
                             stop=True)
            y1 = state.tile([128, NG], f32, tag="hy")
            nc.scalar.activation(y1[:], pm[:, :NG], Act.Identity,
                                 bias=bias_t[:, BCOL_HB1:BCOL_HB1 + 1])
            y2 = head_mm(hwm_t[:, 0, :], y1[:], BCOL_HMID + 0, Act.Relu)
            y3 = head_mm(hwm_t[:, 1, :], y2[:], BCOL_HMID + 1, Act.Identity)
            y4 = head_mm(hwm_t[:, 2, :], y3[:], BCOL_HMID + 2, Act.Relu)
            pm5 = psum_m.tile([128, 128], f32, tag="pm")
            nc.tensor.matmul(pm5[:1, :NG], hw5_t[:], y4[:], start=True,
                             stop=True)
            yout = state.tile([1, NG], f32, tag="yout")
            nc.scalar.activation(yout[:], pm5[:1, :NG], Act.Identity,
                                 bias=bias_t[:1, BCOL_HB5:BCOL_HB5 + 1])
            nc.sync.dma_start(y_d[:], yout[:])

    nc.compile()
    return nc


_NC_CACHE = {}


def _get_program(ns):
    if ns not in _NC_CACHE:
        _NC_CACHE[ns] = _build_program(ns)
    return _NC_CACHE[ns]


def _prep_inputs(inputs):
    """Host-side reachability relabeling + layout prep."""
    inds = np.asarray(inputs["regular_node_inds"]).astype(np.int64)
    shapes = np.asarray(inputs["regular_node_shapes"], dtype=F32)
    edge = np.asarray(inputs["edge_index"]).astype(np.int64)
    last_idx = np.asarray(inputs["last_idx"]).astype(np.int64)

    # --- backward reachability ordering per graph ---
    perms = np.empty((B, N), np.int64)
    sizes = np.zeros((B, 6), np.int64)
    for g in range(B):
        src, dst = edge[g, 0], edge[g, 1]
        in_set = np.zeros(N, bool)
        order = [int(last_idx[g])]
        in_set[order[0]] = True
        sizes[g, 0] = 1
        frontier = np.array(order)
        for k in range(1, 6):
            mask = np.isin(dst, frontier)
            cand = np.unique(src[mask])
            new = cand[~in_set[cand]]
            in_set[new] = True
            order.extend(new.tolist())
            sizes[g, k] = len(order)
            frontier = new
        rest = np.flatnonzero(~in_set)
        perms[g] = np.concatenate([np.array(order, np.int64), rest])

    rup = lambda x: max(128, int(-(-x // 128)) * 128)
    Nk = [rup(int(sizes[:, k].max())) for k in range(6)]
    Nk[0] = 128
    ns = tuple(min(v, N) for v in Nk)
    N0, N1, N2, N3, N4, N5 = ns
    atch = (N5 + 127) // 128

    inv = np.empty((B, N), np.int64)
    for g in range(B):
        inv[g, perms[g]] = np.arange(N)

    # adjacency in relabeled order, only the reachable block
    at_all = np.zeros((B, atch * 128, N4), dtype=F32)
    for g in range(B):
        src_r = inv[g, edge[g, 0]]
        dst_r = inv[g, edge[g, 1]]
        # real (non-padding) dst always has src inside shipped rows by
        # construction; edges dropped here only affect padding-dst values,
        # which no consumer reads
        m = (dst_r < N4) & (src_r < atch * 128)
        np.add.at(at_all[g], (src_r[m], dst_r[m]), 1.0)
    at_all = at_all.reshape(B, atch, 128, N4).astype(FP8)

    # embed rhs: one-hot(inds)^T + shapes^T in relabeled order, prefix N5
    erhs_all = np.zeros((B, 128, N5), dtype=F32)
    ar = np.arange(N5)
    for g in range(B):
        pin = inds[g][perms[g]][:N5]
        erhs_all[g, pin, ar] = 1.0
        erhs_all[g, VOCAB:VOCAB + 4, :] = shapes[g][perms[g]][:N5].T

    embed_w = np.zeros((128, H), dtype=F32)
    embed_w[:VOCAB] = np.asarray(inputs["embed_table"], dtype=F32)
    embed_w[VOCAB:VOCAB + 4] = np.asarray(inputs["shape_w"], dtype=F32)

    aws = np.asarray(inputs["adapt_Wself"], dtype=F32).reshape(L, 2, H, H)
    awn = np.asarray(inputs["adapt_Wnbr"], dtype=F32).reshape(L, 2, H, H)
    aws = np.ascontiguousarray(aws.transpose(0, 2, 1, 3))  # [L, H, 2, H]
    awn = np.ascontiguousarray(awn.transpose(0, 2, 1, 3))
    hw1 = np.ascontiguousarray(
        np.asarray(inputs["hW1"], dtype=F32).reshape(2, H, H).transpose(1, 0, 2))

    biases = np.zeros((H, NBCOL), dtype=F32)
    biases[:, BCOL_BASE:BCOL_BASE + L] = np.asarray(inputs["base_b"], dtype=F32).T
    biases[:, BCOL_ADAPT:BCOL_ADAPT + L] = np.asarray(inputs["adapt_b"], dtype=F32).T
    biases[:, BCOL_HB1] = np.asarray(inputs["hb1"], dtype=F32)
    biases[:, BCOL_HMID:BCOL_HMID + 3] = np.asarray(inputs["hbmid"], dtype=F32).T
    biases[0, BCOL_HB5] = np.asarray(inputs["hb5"], dtype=F32)[0]

    shared = {
        "embed_w": embed_w.astype(BF16),
        "bwself": np.asarray(inputs["base_Wself"], dtype=F32).astype(BF16),
        "bwnbr": np.asarray(inputs["base_Wnbr"], dtype=F32).astype(BF16),
        "awself": aws.astype(BF16),
        "awnbr": awn.astype(BF16),
        "hw1": hw1,
        "hwmid": np.ascontiguousarray(
            np.asarray(inputs["hWmid"], dtype=F32).transpose(1, 0, 2)),
        "hw5": np.asarray(inputs["hW5"], dtype=F32),
        "biases": biases,
    }
    in_maps = []
    for c in range(N_CORES):
        g0 = c * NG
        in_maps.append({
            "at": np.ascontiguousarray(at_all[g0:g0 + NG]),
            "embed_rhs": erhs_all[g0:g0 + NG].astype(BF16),
            **shared,
        })
    return in_maps, ns


def kernel(**inputs) -> np.ndarray:
    in_maps, ns = _prep_inputs(inputs)
    nc = _get_program(ns)
    res = run_bass_kernel_spmd(nc, in_maps, core_ids=list(range(N_CORES)))
    out = np.concatenate([res.results[c]["y"].reshape(NG) for c in range(N_CORES)])
    return out.reshape(B, 1).astype(F32)


# revision 39
# speedup vs baseline: 763769.0000x; 65916.0000x over previous
"""Trainium2 Bass kernel for nn_CGRegressorAdapter (GNN message passing).

Strategy:
  - Data-parallel over B=32 graphs: 8 cores x 4 graphs each. Weights replicated.
  - **Backward reachability slicing**: the readout uses ONE node per graph
    (last_idx), so layer k from the end only needs the k-hop in-neighborhood
    S_k of that node (|S_k| ~ 1, 9, 39, 152, 562, 1390 here). Nodes are
    relabeled per graph on the host so each S_k is a PREFIX; every matmul
    then runs on prefix widths only (exact computation, ~13x fewer
    aggregation columns than the full dense form). Prefix widths are padded
    to shared maxima so all 8 cores run one SPMD program; values computed in
    the padding region are garbage-but-unread by construction.
  - Per-graph dense adjacency AT[src, dst] (edge-count matrix) built on host
    in relabeled order, shipped fp8 e4m3 (counts <= 2, exact), only the
    [S5-rows x S4-cols] block that aggregation can touch.
  - All states/weights bf16 (PSUM accumulates f32; rel err ~7e-3 vs the 2e-2
    gate). Moving operands never f32 (f32 moving streams at 1/4 rate).
  - GraphConv: m = h @ Wnbr into PSUM, DVE-cast to bf16; agg^T accumulated
    as m_chunk @ AT rows over the dst prefix in <=512-col PSUM spans, plus
    the Wself path in the same accumulation group; fused bias+ReLU on ACT
    writes the next bf16 state.
  - Readout: relabeled last node is column 0 - extraction is a [128,1] copy.
  - Small regression head entirely on-chip in f32.
"""
import numpy as np
import ml_dtypes

import concourse.bass as bass
import concourse.mybir as mybir
from concourse import bacc
from concourse.bass import ts
from concourse.bass_utils import run_bass_kernel_spmd
from concourse.tile import TileContext

BF16 = ml_dtypes.bfloat16
FP8 = ml_dtypes.float8_e4m3
F32 = np.float32

B, N, E, H, L, VOCAB = 32, 2048, 8192, 128, 4, 32
N_CORES = 8
NG = B // N_CORES          # graphs per core
dt = mybir.dt
Alu = mybir.AluOpType
Act = mybir.ActivationFunctionType

# bias column indices in the packed bias tile
BCOL_BASE = 0      # 0..3  base_b
BCOL_ADAPT = 4     # 4..7  adapt_b
BCOL_HB1 = 8
BCOL_HMID = 9      # 9..11
BCOL_HB5 = 12
NBCOL = 16


def _spans(w):
    """Split width w into <=512-wide PSUM span pieces: [(span, off, width)]."""
    out = []
    off = 0
    while off < w:
        out.append((off // 512, off, min(512, w - off)))
        off += 512
    return out


def _build_program(ns, n_graphs=NG, n_repeat=1):
    """ns = (N0..N5): padded prefix sizes, multiples of 128, N5 = embed width."""
    N0, N1, N2, N3, N4, N5 = ns
    cdiv = lambda x: (x + 127) // 128
    atch = cdiv(N5)          # adjacency src chunks shipped
    atw = N4                 # adjacency dst width shipped
    # (dst_width, src_chunks) per gconv
    base_cfg = [(N4, cdiv(N5)), (N3, cdiv(N4)), (N2, cdiv(N3)), (N1, cdiv(N2))]
    adapt_cfg = [(N3, cdiv(N4)), (N2, cdiv(N3)), (N1, cdiv(N2)), (N0, cdiv(N1))]

    nc = bacc.Bacc("TRN2", target_bir_lowering=False, debug=False,
                   num_devices=N_CORES)
    f32, bf16, fp8 = dt.float32, dt.bfloat16, dt.float8e4

    at_d = nc.declare_dram_parameter("at", [NG, atch, 128, atw], fp8,
                                     isOutput=False)
    erhs_d = nc.declare_dram_parameter("embed_rhs", [NG, 128, N5], bf16,
                                       isOutput=False)
    embw_d = nc.declare_dram_parameter("embed_w", [128, H], bf16, isOutput=False)
    bws_d = nc.declare_dram_parameter("bwself", [L, H, H], bf16, isOutput=False)
    bwn_d = nc.declare_dram_parameter("bwnbr", [L, H, H], bf16, isOutput=False)
    aws_d = nc.declare_dram_parameter("awself", [L, H, 2, H], bf16, isOutput=False)
    awn_d = nc.declare_dram_parameter("awnbr", [L, H, 2, H], bf16, isOutput=False)
    hw1_d = nc.declare_dram_parameter("hw1", [H, 2, H], f32, isOutput=False)
    hwm_d = nc.declare_dram_parameter("hwmid", [H, 3, H], f32, isOutput=False)
    hw5_d = nc.declare_dram_parameter("hw5", [H, 1], f32, isOutput=False)
    bias_d = nc.declare_dram_parameter("biases", [H, NBCOL], f32, isOutput=False)
    y_d = nc.declare_dram_parameter("y", [1, NG], f32, isOutput=True)

    with TileContext(nc) as tc:
        with (
            tc.tile_pool(name="const", bufs=1) as const,
            tc.tile_pool(name="atp", bufs=2) as atp,
            tc.tile_pool(name="state", bufs=1) as state,
            tc.tile_pool(name="currp", bufs=2) as currp,
            tc.tile_pool(name="mp", bufs=4) as mp,
            tc.tile_pool(name="work", bufs=2) as work,
            tc.tile_pool(name="psum_agg", bufs=1, space="PSUM") as psum_agg,
            tc.tile_pool(name="psum_m", bufs=4, space="PSUM") as psum_m,
        ):
            # ---- constants (critical-path first: embed + layer-0 weights) ----
            embw = const.tile([128, H], bf16)
            nc.sync.dma_start(embw[:], embw_d[:])
            bias_t = const.tile([H, NBCOL], f32)
            nc.sync.dma_start(bias_t[:], bias_d[:])
            bws_t = [const.tile([H, H], bf16, tag=f"bws{i}", name=f"bws{i}")
                     for i in range(L)]
            bwn_t = [const.tile([H, H], bf16, tag=f"bwn{i}", name=f"bwn{i}")
                     for i in range(L)]
            aws_t = [const.tile([H, 2, H], bf16, tag=f"aws{i}", name=f"aws{i}")
                     for i in range(L)]
            awn_t = [const.tile([H, 2, H], bf16, tag=f"awn{i}", name=f"awn{i}")
                     for i in range(L)]
            nc.sync.dma_start(bwn_t[0][:], bwn_d[0])
            nc.sync.dma_start(bws_t[0][:], bws_d[0])
            hw1_t = const.tile([H, 2, H], f32)
            hwm_t = const.tile([H, 3, H], f32)
            hw5_t = const.tile([H, 1], f32)

            def load_late_consts():
                for i in range(L):
                    if i > 0:
                        nc.sync.dma_start(bws_t[i][:], bws_d[i])
                        nc.sync.dma_start(bwn_t[i][:], bwn_d[i])
                    nc.sync.dma_start(aws_t[i][:], aws_d[i])
                    nc.sync.dma_start(awn_t[i][:], awn_d[i])
                nc.sync.dma_start(hw1_t[:], hw1_d[:])
                nc.sync.dma_start(hwm_t[:], hwm_d[:])
                nc.sync.dma_start(hw5_t[:], hw5_d[:])

            gbT = state.tile([128, NG], f32, tag="gb")
            gaT = state.tile([128, NG], f32, tag="ga")

            def gconv(nbr_srcs, self_srcs, at_tiles, bias_col, out_tile,
                      dst_w, src_chunks):
                """All operands bf16. Aggregates over dst prefix [0, dst_w)
                from src chunks [0, src_chunks)."""
                pieces = _spans(dst_w)
                aggs = {s: psum_agg.tile([128, 512], f32, tag=f"agg{s}",
                                         name=f"agg{s}")
                        for s, _, _ in pieces}

                def emit_m(j):
                    pm = psum_m.tile([128, 128], f32, tag="pm")
                    nlast = len(nbr_srcs) - 1
                    for idx, (src, w) in enumerate(nbr_srcs):
                        nc.tensor.matmul(pm[:], src[:, ts(j, 128)], w,
                                         start=(idx == 0), stop=(idx == nlast))
                    mhi = mp.tile([128, 128], bf16, tag="mhi")
                    nc.vector.tensor_copy(out=mhi[:], in_=pm[:])
                    return mhi

                m_next = emit_m(0)
                for idx, (src, w) in enumerate(self_srcs):
                    for s, off, wd in pieces:
                        nc.tensor.matmul(aggs[s][:, :wd], w,
                                         src[:, off:off + wd],
                                         start=(idx == 0), stop=False)
                for j in range(src_chunks):
                    mhi = m_next
                    if j + 1 < src_chunks:
                        m_next = emit_m(j + 1)
                    for s, off, wd in pieces:
                        nc.tensor.matmul(aggs[s][:, :wd], mhi[:],
                                         at_tiles[j][:, off:off + wd],
                                         start=False,
                                         stop=(j == src_chunks - 1))
                for s, off, wd in pieces:
                    nc.scalar.activation(out_tile[:, off:off + wd],
                                         aggs[s][:, :wd], Act.Relu,
                                         bias=bias_t[:, bias_col:bias_col + 1])

            first = True
            for g in list(range(n_graphs)) * n_repeat:
                # ---- per-graph data DMAs ----
                erhs = work.tile([128, N5], bf16, tag="erhs")
                nc.sync.dma_start(erhs[:], erhs_d[g])
                at_t = []
                for j in range(atch):
                    t = atp.tile([128, atw], fp8, tag=f"at{j}", name=f"at{j}")
                    nc.sync.dma_start(t[:], at_d[g, j])
                    at_t.append(t)
                if first:
                    load_late_consts()
                    first = False

                lat = [state.tile([128, N5], bf16, tag=f"lat{k}",
                                  name=f"lat{k}") for k in range(L + 1)]
                for s, off, wd in _spans(N5):
                    eagg = psum_agg.tile([128, 512], f32, tag=f"agg{s}",
                                         name=f"eagg{s}")
                    nc.tensor.matmul(eagg[:, :wd], embw[:],
                                     erhs[:, off:off + wd],
                                     start=True, stop=True)
                    nc.scalar.copy(lat[0][:, off:off + wd], eagg[:, :wd])

                # ---- base stack ----
                for i in range(L):
                    dw, sc = base_cfg[i]
                    gconv([(lat[i], bwn_t[i][:])], [(lat[i], bws_t[i][:])],
                          at_t, BCOL_BASE + i, lat[i + 1], dw, sc)

                # ---- adapter stack ----
                curr = lat[0]
                for i in range(L):
                    dw, sc = adapt_cfg[i]
                    ncurr = currp.tile([128, N3], bf16, tag="curr")
                    gconv([(lat[i + 1], awn_t[i][:, 0, :]),
                           (curr, awn_t[i][:, 1, :])],
                          [(lat[i + 1], aws_t[i][:, 0, :]),
                           (curr, aws_t[i][:, 1, :])],
                          at_t, BCOL_ADAPT + i, ncurr, dw, sc)
                    curr = ncurr

                # ---- readout: relabeled last node is column 0 ----
                nc.vector.tensor_copy(out=gbT[:, g:g + 1], in_=lat[L][:, 0:1])
                nc.vector.tensor_copy(out=gaT[:, g:g + 1], in_=curr[:, 0:1])

            # ---- regression head (all graphs at once) ----
            def head_mm(lhsT, rhs, bias_col, func):
                pm = psum_m.tile([128, 128], f32, tag="pm")
                nc.tensor.matmul(pm[:, :NG], lhsT, rhs, start=True, stop=True)
                out = state.tile([128, NG], f32, tag="hy")
                nc.scalar.activation(out[:], pm[:, :NG], func,
                                     bias=bias_t[:, bias_col:bias_col + 1])
                return out

            pm = psum_m.tile([128, 128], f32, tag="pm")
            nc.tensor.matmul(pm[:, :NG], hw1_t[:, 0, :], gbT[:], start=True,
                             stop=False)
            nc.tensor.matmul(pm[:, :NG], hw1_t[:, 1, :], gaT[:], start=False,
                             stop=True)
            y1 = state.tile([128, NG], f32, tag="hy")
            nc.scalar.activation(y1[:], pm[:, :NG], Act.Identity,
                                 bias=bias_t[:, BCOL_HB1:BCOL_HB1 + 1])
            y2 = head_mm(hwm_t[:, 0, :], y1[:], BCOL_HMID + 0, Act.Relu)
            y3 = head_mm(hwm_t[:, 1, :], y2[:], BCOL_HMID + 1, Act.Identity)
            y4 = head_mm(hwm_t[:, 2, :], y3[:], BCOL_HMID + 2, Act.Relu)
            pm5 = psum_m.tile([128, 128], f32, tag="pm")
            nc.tensor.matmul(pm5[:1, :NG], hw5_t[:], y4[:], start=True,
                             stop=True)
            yout = state.tile([1, NG], f32, tag="yout")
            nc.scalar.activation(yout[:], pm5[:1, :NG], Act.Identity,
                                 bias=bias_t[:1, BCOL_HB5:BCOL_HB5 + 1])
            nc.sync.dma_start(y_d[:], yout[:])

    nc.compile()
    return nc


_NC_CACHE = {}


def _get_program(ns):
    if ns not in _NC_CACHE:
        _NC_CACHE[ns] = _build_program(ns)
    return _NC_CACHE[ns]


def _prep_inputs(inputs):
    """Host-side reachability relabeling + layout prep."""
    inds = np.asarray(inputs["regular_node_inds"]).astype(np.int64)
    shapes = np.asarray(inputs["regular_node_shapes"], dtype=F32)
    edge = np.asarray(inputs["edge_index"]).astype(np.int64)
    last_idx = np.asarray(inputs["last_idx"]).astype(np.int64)

    # --- backward reachability ordering per graph ---
    perms = np.empty((B, N), np.int64)
    sizes = np.zeros((B, 6), np.int64)
    for g in range(B):
        src, dst = edge[g, 0], edge[g, 1]
        in_set = np.zeros(N, bool)
        order = [int(last_idx[g])]
        in_set[order[0]] = True
        sizes[g, 0] = 1
        frontier = np.array(order)
        for k in range(1, 6):
            mask = np.isin(dst, frontier)
            cand = np.unique(src[mask])
            new = cand[~in_set[cand]]
            in_set[new] = True
            order.extend(new.tolist())
            sizes[g, k] = len(order)
            frontier = new
        rest = np.flatnonzero(~in_set)
        perms[g] = np.concatenate([np.array(order, np.int64), rest])

    rup = lambda x: max(128, int(-(-x // 128)) * 128)
    Nk = [rup(int(sizes[:, k].max())) for k in range(6)]
    Nk[0] = 128
    ns = tuple(min(v, N) for v in Nk)
    N0, N1, N2, N3, N4, N5 = ns
    atch = (N5 + 127) // 128

    inv = np.empty((B, N), np.int64)
    for g in range(B):
        inv[g, perms[g]] = np.arange(N)

    # adjacency in relabeled order, only the reachable block
    at_all = np.zeros((B, atch * 128, N4), dtype=F32)
    for g in range(B):
        src_r = inv[g, edge[g, 0]]
        dst_r = inv[g, edge[g, 1]]
        # real (non-padding) dst always has src inside shipped rows by
        # construction; edges dropped here only affect padding-dst values,
        # which no consumer reads
        m = (dst_r < N4) & (src_r < atch * 128)
        np.add.at(at_all[g], (src_r[m], dst_r[m]), 1.0)
    at_all = at_all.reshape(B, atch, 128, N4).astype(FP8)

    # embed rhs: one-hot(inds)^T + shapes^T in relabeled order, prefix N5
    erhs_all = np.zeros((B, 128, N5), dtype=F32)
    ar = np.arange(N5)
    for g in range(B):
        pin = inds[g][perms[g]][:N5]
        erhs_all[g, pin, ar] = 1.0
        erhs_all[g, VOCAB:VOCAB + 4, :] = shapes[g][perms[g]][:N5].T

    embed_w = np.zeros((128, H), dtype=F32)
    embed_w[:VOCAB] = np.asarray(inputs["embed_table"], dtype=F32)
    embed_w[VOCAB:VOCAB + 4] = np.asarray(inputs["shape_w"], dtype=F32)

    aws = np.asarray(inputs["adapt_Wself"], dtype=F32).reshape(L, 2, H, H)
    awn = np.asarray(inputs["adapt_Wnbr"], dtype=F32).reshape(L, 2, H, H)
    aws = np.ascontiguousarray(aws.transpose(0, 2, 1, 3))  # [L, H, 2, H]
    awn = np.ascontiguousarray(awn.transpose(0, 2, 1, 3))
    hw1 = np.ascontiguousarray(
        np.asarray(inputs["hW1"], dtype=F32).reshape(2, H, H).transpose(1, 0, 2))

    biases = np.zeros((H, NBCOL), dtype=F32)
    biases[:, BCOL_BASE:BCOL_BASE + L] = np.asarray(inputs["base_b"], dtype=F32).T
    biases[:, BCOL_ADAPT:BCOL_ADAPT + L] = np.asarray(inputs["adapt_b"], dtype=F32).T
    biases[:, BCOL_HB1] = np.asarray(inputs["hb1"], dtype=F32)
    biases[:, BCOL_HMID:BCOL_HMID + 3] = np.asarray(inputs["hbmid"], dtype=F32).T
    biases[0, BCOL_HB5] = np.asarray(inputs["hb5"], dtype=F32)[0]

    shared = {
        "embed_w": embed_w.astype(BF16),
        "bwself": np.asarray(inputs["base_Wself"], dtype=F32).astype(BF16),
        "bwnbr": np.asarray(inputs["base_Wnbr"], dtype=F32).astype(BF16),
        "awself": aws.astype(BF16),
        "awnbr": awn.astype(BF16),
        "hw1": hw1,
        "hwmid": np.ascontiguousarray(
            np.asarray(inputs["hWmid"], dtype=F32).transpose(1, 0, 2)),
        "hw5": np.asarray(inputs["hW5"], dtype=F32),
        "biases": biases,
    }
    in_maps = []
    for c in range(N_CORES):
        g0 = c * NG
        in_maps.append({
            "at": np.ascontiguousarray(at_all[g0:g0 + NG]),
            "embed_rhs": erhs_all[g0:g0 + NG].astype(BF16),
            **shared,
        })
    return in_maps, ns


def kernel(**inputs) -> np.ndarray:
    in_maps, ns = _prep_inputs(inputs)
    nc = _get_program(ns)
    res = run_bass_kernel_spmd(nc, in_maps, core_ids=list(range(N_CORES)))
    out = np.concatenate([res.results[c]["y"].reshape(NG) for c in range(N_CORES)])
    return out.reshape(B, 1).astype(F32)
